# revision 30
# baseline (speedup 1.0000x reference)
"""Trainium2 Bass kernel: batched Ising energies E_b = s_b^T J s_b.

state: [1024, 2048] float32 in {0,1};  J: [2048, 2048] float32.
Returns energies [1024] float32.

Strategy (8 NeuronCores): symmetric-half circulant decomposition.
With A = J + J^T, E_b = 1/2 s_b^T A s_b needs only the 136 distinct
128x128 block-pairs {(p,q): p<=q} of the 16x16 block grid instead of
all 256.  A rotational starter splits those pairs into 4 isomorphic
34-block templates: core c (c = 0..3) owns block-columns
{c, 4+c, 8+c, 12+c}; column 4k+c accumulates contributions from
p = (4k+c+d) mod 16 for d = 0..7 (+ d = 8 for k = 0,1), i.e. 9/9/8/8
blocks.  Every core therefore runs the IDENTICAL instruction stream;
only the data (J blocks, spin-block permutation) differs.  The batch
is halved across the remaining factor of 2 (8 = 4 templates x 2).

Per-core compute: for column q, ps[qcol, b] = sum_p W_pq^T spins_p
via 8-9 accumulating PE matmuls (lhsT = 128x128 J block, rhs =
spins_p^T [128, 512 samples], full 512 moving dim).  Epilogue:
m = ps * spins_q^T elementwise on the vector engine (spins^T is the
same layout as the streamed state, so no second spin tensor is
shipped), then a ones-vector matmul folds the 128 partition rows into
a [1, 512] per-sample partial.  One 2KB output DMA per core; the host
sums the 4 template partials per batch half.

Scheduling notes (from perfetto traces):
 - each dma_start costs ~650 ns of DIRECT2D descriptor-write on its
   issuing sequencer, so the loads are split across TWO sequencers
   (su on gpsimd, J on sync) with small first chunks: the first
   matmul starts ~2 us earlier and the PE is never descriptor-gated.
 - uint8 -> +-1 bf16 spin expansion is split between the vector and
   scalar engines so it always stays ahead of the matmul stream.
 - the column-k reduce matmul is deferred several J matmuls so the PE
   never stalls on the vector multiply.
 - TileContext's stock teardown zeroes ~250 semaphores serially on
   gpsimd (~8 us!); _FastTeardown splits the sem_clear range across
   all five engines (~1.5 us), keeping the same drain + barriers +
   DMA-queue reset semantics.
"""

import sys

if "/opt/trn_rl_repo" not in sys.path:
    sys.path.insert(0, "/opt/trn_rl_repo")

import numpy as np
import ml_dtypes

B, N = 1024, 2048
P = 128
NB = N // P          # 16 spin blocks
NT = 4               # templates (J-column groups)
C = 2                # batch halves
BH = B // C          # 512 samples per core
NBLK = 34            # J blocks per core
N_WARM = 40          # dummy matmuls to warm the PE clock gate
WARM_N = 64          # free dim of each warmup matmul

# template: per column k (q = 4k+c), the d-offsets of contributing blocks
_COL_DS = [
    [0, 1, 2, 3, 4, 5, 6, 7, 8],   # k=0: 9 blocks
    [0, 1, 2, 3, 4, 5, 6, 7, 8],   # k=1: 9 blocks
    [0, 1, 2, 3, 4, 5, 6, 7],      # k=2: 8 blocks
    [0, 1, 2, 3, 4, 5, 6, 7],      # k=3: 8 blocks
]
# per-block (col k, local su slot lam = (4k+d) mod 16), in stream order
_BLOCKS = [
    (k, (4 * k + d) % NB) for k in range(4) for d in _COL_DS[k]
]
_COL_END = []  # index of last block of each column
for k in range(4):
    _COL_END.append(max(i for i, (kk, _) in enumerate(_BLOCKS) if kk == k))

_cache = {}


def _build_program():
    import concourse.bacc as bacc
    import concourse.mybir as mybir
    import concourse.tile as tile
    from concourse.bass import compact_to_ranges
    from concourse.vector_clock import ScopedClock

    bf16 = mybir.dt.bfloat16
    f32 = mybir.dt.float32
    f32r = mybir.dt.float32r
    u8 = mybir.dt.uint8

    class _FastTeardown(tile.TileContext):
        """Stock teardown sem_clears ~250 sems serially on gpsimd
        (~8 us).  Same semantics, but the clears are spread across all
        five engines and run concurrently between the two barriers."""

        def _drain_and_barrier(self, tick_clock, wait_clock):
            nc = self.nc
            drain_inst = nc.sync.drain()
            wait_clock.add_sem_waits(
                drain_inst.ins, ScopedClock({None: tick_clock.global_clock})
            )
            nc.all_engine_barrier()
            popped = nc._tile_sem_poison_stack.pop()
            assert popped is self._sem_poison
            sems = list(self.sems.allocated().values())
            sem_nums = [
                s.num if hasattr(s, "num") else int(s) for s in sems
            ]
            ranges = compact_to_ranges(sem_nums)
            for r in ranges:
                assert nc._state.free_isdisjoint(r)
                nc.gpsimd.dma_reset(r)
            # sem_clear lowers to a per-semaphore EVENT_SEMAPHORE train
            # (~115ns per sem, ~6us for the full window); the hardware
            # EVENT_SEMAPHORE_RANGE_CLEAR op zeroes a whole range in one
            # sequencer instruction instead.
            mode = nc.isa.get_enum(
                "NEURON_ISA_TPB_EVENT_SEMAPHORE_CLEAR_MODE"
            ).NEURON_ISA_TPB_EVENT_SEMAPHORE_CLEAR_MODE_SEMAPHORE_ZERO.value
            engines = [nc.gpsimd, nc.sync, nc.scalar, nc.vector, nc.tensor]
            for i, r in enumerate(ranges):
                engines[i % len(engines)].isa(
                    nc.isa.Opcode.NEURON_ISA_TPB_OPCODE_EVENT_SEMAPHORE_RANGE_CLEAR,
                    {
                        "mode": mode,
                        "range_first": r.start,
                        "range_last": r.stop - 1,
                    },
                )
            nc._state.prepend_free_semaphores(sem_nums)
            for poison_set in nc._tile_sem_poison_stack:
                poison_set.update(sem_nums)
            # no second barrier: the engines' own end-of-program epilogues
            # (emitted by the backend) already serialize behind this block
            nc.all_engine_barrier(sem_only=True)

    nc = bacc.Bacc("TRN2", target_bir_lowering=False, debug=False,
                   num_devices=NT * C)

    # spins ship hybrid: slots 0-8 as uint8 (half the bytes; expanded on
    # the vector engine which is otherwise idle early), slots 9-15 as
    # bf16 (no expansion dependency for the late slots)
    su_ext = nc.dram_tensor("su", [P, 9 * BH], u8, kind="ExternalInput").ap()
    sh_ext = nc.dram_tensor("sh", [P, 7 * BH], bf16, kind="ExternalInput").ap()
    jb_ext = nc.dram_tensor("jb", [P, NBLK * P], bf16, kind="ExternalInput").ap()
    # full-partition output (all 128 rows carry the same per-sample
    # partial): tiny single-partition DMAs take a ~10us slow path at the
    # end of a busy kernel, full 128-partition descriptors do not
    OW = 32  # output replication width (full-partition-class DMA, 64KB)
    out_ext = nc.dram_tensor("part", [OW, BH], f32, kind="ExternalOutput").ap()

    with _FastTeardown(nc) as tc:
        with (
            tc.tile_pool(name="persist", bufs=1) as persist,
            tc.tile_pool(name="work", bufs=2) as work,
            tc.tile_pool(name="psum", bufs=1, space="PSUM") as psum_pool,
            tc.tile_pool(name="warmps", bufs=1, space="PSUM") as warm_pool,
        ):
            su_t = persist.tile([P, 9, BH], u8)
            st_t = persist.tile([P, NB, BH], bf16)
            jb_t = persist.tile([P, NBLK, P], bf16)
            ones = persist.tile([P, OW], f32r)
            red_sb = persist.tile([OW, BH], f32)
            warm_src = persist.tile([P, P], bf16)

            nc.vector.memset(warm_src[:], 0.0)
            # f32r memset trips an ISA check; synthesize 1.0s on the DVE
            nc.vector.tensor_scalar(
                ones[:], warm_src[:, :OW], 0.0, 1.0,
                mybir.AluOpType.mult, mybir.AluOpType.add,
            )

            # PE warmup against the HAM clock gate while loads stream in
            warm_ps = warm_pool.tile([P, WARM_N], f32)
            for _ in range(N_WARM):
                nc.tensor.matmul(
                    warm_ps, lhsT=warm_src[:], rhs=warm_src[:, :WARM_N],
                    start=True, stop=True,
                )

            # Input loads on two rings in consumption order (both rings
            # share ~390GB/s once streaming; each dma_start costs ~650ns
            # of descriptor write on its sequencer, so chunks are few with
            # small heads).  The scalar engine does no compute, so its
            # ring has no ACT-table preamble and starts immediately.
            su3 = su_ext.rearrange("p (k b) -> p k b", b=BH)
            sh3 = sh_ext.rearrange("p (k b) -> p k b", b=BH)
            jb3 = jb_ext.rearrange("p (j c) -> p j c", c=P)

            nc.sync.dma_start(out=su_t[:, 0:2], in_=su3[:, 0:2])
            nc.scalar.dma_start(out=jb_t[:, 0:3], in_=jb3[:, 0:3])
            nc.sync.dma_start(out=su_t[:, 2:5], in_=su3[:, 2:5])
            nc.scalar.dma_start(out=jb_t[:, 3:9], in_=jb3[:, 3:9])
            nc.sync.dma_start(out=su_t[:, 5:9], in_=su3[:, 5:9])
            nc.scalar.dma_start(out=jb_t[:, 9:18], in_=jb3[:, 9:18])
            nc.sync.dma_start(out=st_t[:, 9:16], in_=sh3[:, 0:7])
            nc.scalar.dma_start(out=jb_t[:, 18:34], in_=jb3[:, 18:34])

            # expand uint8 {0,1} -> +-1.0 bf16 spins for slots 0-8
            for sl in range(9):
                nc.vector.tensor_scalar(
                    st_t[:, sl], su_t[:, sl], 2.0, -1.0,
                    mybir.AluOpType.mult, mybir.AluOpType.add,
                )

            ps_cols = [
                psum_pool.tile([P, BH], f32, name=f"ps_{k}") for k in range(4)
            ]
            ps_red = psum_pool.tile([OW, BH], f32, name="ps_red")

            # J matmuls with epilogues interleaved.  The column-k reduce
            # matmul is deferred so the PE never stalls on the DVE multiply.
            red_at = {20: 0, 26: 1, 31: 2}
            m_tiles = {}

            def epilogue_mul(k):
                m = work.tile([P, BH], f32r, name="m_col")
                nc.vector.scalar_tensor_tensor(
                    m[:],
                    ps_cols[k][:],
                    1.0,
                    st_t[:, 4 * k],
                    mybir.AluOpType.mult,
                    mybir.AluOpType.mult,
                )
                m_tiles[k] = m

            def red_mm(k):
                nc.tensor.matmul(
                    ps_red,
                    lhsT=ones[:],
                    rhs=m_tiles[k][:],
                    start=(k == 0),
                    stop=(k == 3),
                )

            # Tiny filler matmuls between the early real matmuls keep the
            # PE busy while the first su/jb chunks stream in — gaps there
            # would keep the HAM clock gate at 1.2 GHz for several extra
            # microseconds.
            fillers = {0: 6, 1: 8, 2: 8, 3: 6, 4: 4, 5: 3, 6: 2}

            seen_start = set()
            for j, (k, lam) in enumerate(_BLOCKS):
                nc.tensor.matmul(
                    ps_cols[k],
                    lhsT=jb_t[:, j],
                    rhs=st_t[:, lam],
                    start=(k not in seen_start),
                    stop=(j == _COL_END[k]),
                )
                seen_start.add(k)
                for _ in range(fillers.get(j, 0)):
                    nc.tensor.matmul(
                        warm_ps, lhsT=warm_src[:], rhs=warm_src[:, :WARM_N],
                        start=True, stop=True,
                    )
                if j in _COL_END:
                    epilogue_mul(_COL_END.index(j))
                if j in red_at:
                    red_mm(red_at[j])
            red_mm(3)

            # copy on the vector engine (keeping the scalar engine
            # compute-free avoids its ACT table preamble load entirely);
            # the out DMA issues from the same queue so no cross-engine
            # semaphore hop sits on the critical tail
            nc.vector.tensor_scalar(
                red_sb[:], ps_red[:], 1.0, 0.0,
                mybir.AluOpType.mult, mybir.AluOpType.add,
            )
            nc.sync.dma_start(out=out_ext, in_=red_sb[:])

    nc.compile()
    return nc


def _make_in_maps(state, J):
    bf16 = ml_dtypes.bfloat16
    state = np.asarray(state, dtype=np.float32)
    J = np.asarray(J, dtype=np.float32)

    u_all = state.astype(np.uint8)                   # [B, N] {0,1}
    uT = np.ascontiguousarray(u_all.T).reshape(NB, P, B)
    spins = (state * 2.0 - 1.0).astype(bf16)         # [B, N] exact +-1
    sT = np.ascontiguousarray(spins.T).reshape(NB, P, B)
    A = J + J.T                                      # symmetrized, fp32
    Ab = A.reshape(NB, P, NB, P)

    # J blocks per template c: [34, 128, 128] -> [128, 34*128] bf16
    jb_by_c = []
    for c in range(NT):
        blocks = np.empty((NBLK, P, P), dtype=np.float32)
        for j, (k, lam) in enumerate(_BLOCKS):
            q = (4 * k + c) % NB
            p = (lam + c) % NB
            w = 0.5 if p == q else 1.0
            blocks[j] = Ab[p, :, q, :] * w
        jb_by_c.append(
            np.ascontiguousarray(
                blocks.transpose(1, 0, 2).reshape(P, NBLK * P)
            ).astype(bf16)
        )

    in_maps = []
    placement = []
    for core in range(NT * C):
        c, h = divmod(core, C)
        # local slot lam holds global spin block (lam + c) mod 16;
        # slots 0-8 ship uint8, slots 9-15 ship bf16
        perm = [(lam + c) % NB for lam in range(NB)]
        hs = slice(h * BH, (h + 1) * BH)
        su = np.ascontiguousarray(
            uT[perm[:9]][:, :, hs].transpose(1, 0, 2).reshape(P, 9 * BH)
        )
        sh = np.ascontiguousarray(
            sT[perm[9:]][:, :, hs].transpose(1, 0, 2).reshape(P, 7 * BH)
        )
        in_maps.append({"su": su, "sh": sh, "jb": jb_by_c[c]})
        placement.append((c, h))
    return in_maps, placement


def kernel(state, J):
    from concourse.bass_utils import run_bass_kernel_spmd

    if "nc" not in _cache:
        _cache["nc"] = _build_program()
    nc = _cache["nc"]

    in_maps, placement = _make_in_maps(state, J)
    res = run_bass_kernel_spmd(nc, in_maps, list(range(NT * C)))

    out = np.zeros(B, dtype=np.float32)
    for core, (c, h) in enumerate(placement):
        out[h * BH:(h + 1) * BH] += res.results[core]["part"][0]
    return out


# revision 31
# speedup vs baseline: 1.0062x; 1.0062x over previous
"""Trainium2 Bass kernel: batched Ising energies E_b = s_b^T J s_b.

state: [1024, 2048] float32 in {0,1};  J: [2048, 2048] float32.
Returns energies [1024] float32.

Strategy (8 NeuronCores): symmetric-half circulant decomposition.
With A = J + J^T, E_b = 1/2 s_b^T A s_b needs only the 136 distinct
128x128 block-pairs {(p,q): p<=q} of the 16x16 block grid instead of
all 256.  A rotational starter splits those pairs into 4 isomorphic
34-block templates: core c (c = 0..3) owns block-columns
{c, 4+c, 8+c, 12+c}; column 4k+c accumulates contributions from
p = (4k+c+d) mod 16 for d = 0..7 (+ d = 8 for k = 0,1), i.e. 9/9/8/8
blocks.  Every core therefore runs the IDENTICAL instruction stream;
only the data (J blocks, spin-block permutation) differs.  The batch
is halved across the remaining factor of 2 (8 = 4 templates x 2).

Per-core compute: for column q, ps[qcol, b] = sum_p W_pq^T spins_p
via 8-9 accumulating PE matmuls (lhsT = 128x128 J block, rhs =
spins_p^T [128, 512 samples], full 512 moving dim).  Epilogue:
m = ps * spins_q^T elementwise on the vector engine (spins^T is the
same layout as the streamed state, so no second spin tensor is
shipped), then a ones-vector matmul folds the 128 partition rows into
a [1, 512] per-sample partial.  One 2KB output DMA per core; the host
sums the 4 template partials per batch half.

Scheduling notes (from perfetto traces):
 - each dma_start costs ~650 ns of DIRECT2D descriptor-write on its
   issuing sequencer and the rings deliver ~390 GB/s aggregate after a
   ~1.5 us pipe-fill, so loads ride TWO rings in consumption order
   (spins on sync, J blocks on scalar) with small head chunks; the
   scalar engine does no compute so its ring has no ACT-table preamble.
 - spins ship hybrid: slots 0-8 as uint8 (half the bytes, expanded to
   +-1 bf16 on the otherwise-idle vector engine), slots 9-15 as bf16
   (no expansion dependency for the late slots).
 - tiny filler matmuls between the early real matmuls keep the PE's
   HAM activity window busy while the first chunks stream in;
   otherwise the clock gate holds the PE at 1.2 GHz for ~8 us.
 - the column-k reduce matmul is deferred several J matmuls so the PE
   never stalls on the vector multiply.
 - output is replicated over 32 partitions: single-partition or
   few-bytes-per-partition output DMAs take a ~10 us slow path at the
   end of a busy kernel, wide descriptors do not.
 - TileContext's stock teardown clears each semaphore individually on
   gpsimd (~115 ns/sem, ~6 us); _FastTeardown issues hardware
   EVENT_SEMAPHORE_RANGE_CLEAR ops instead and ends with a sem-only
   barrier.  The remaining ~6 us tail is the backend (walrus) per-
   engine epilogue, which is not reachable from the bass level.
"""

import sys

if "/opt/trn_rl_repo" not in sys.path:
    sys.path.insert(0, "/opt/trn_rl_repo")

import numpy as np
import ml_dtypes

B, N = 1024, 2048
P = 128
NB = N // P          # 16 spin blocks
NT = 4               # templates (J-column groups)
C = 2                # batch halves
BH = B // C          # 512 samples per core
NBLK = 34            # J blocks per core
N_WARM = 40          # dummy matmuls to warm the PE clock gate
WARM_N = 64          # free dim of each warmup matmul

# template: per column k (q = 4k+c), the d-offsets of contributing blocks
_COL_DS = [
    [0, 1, 2, 3, 4, 5, 6, 7, 8],   # k=0: 9 blocks
    [0, 1, 2, 3, 4, 5, 6, 7, 8],   # k=1: 9 blocks
    [0, 1, 2, 3, 4, 5, 6, 7],      # k=2: 8 blocks
    [0, 1, 2, 3, 4, 5, 6, 7],      # k=3: 8 blocks
]
# per-block (col k, local su slot lam = (4k+d) mod 16), in stream order
_BLOCKS = [
    (k, (4 * k + d) % NB) for k in range(4) for d in _COL_DS[k]
]
_COL_END = []  # index of last block of each column
for k in range(4):
    _COL_END.append(max(i for i, (kk, _) in enumerate(_BLOCKS) if kk == k))

_cache = {}


def _build_program():
    import concourse.bacc as bacc
    import concourse.mybir as mybir
    import concourse.tile as tile
    from concourse.bass import compact_to_ranges
    from concourse.vector_clock import ScopedClock

    bf16 = mybir.dt.bfloat16
    f32 = mybir.dt.float32
    f32r = mybir.dt.float32r
    u8 = mybir.dt.uint8

    class _FastTeardown(tile.TileContext):
        """Stock teardown sem_clears ~250 sems serially on gpsimd
        (~8 us).  Same semantics, but the clears are spread across all
        five engines and run concurrently between the two barriers."""

        def _drain_and_barrier(self, tick_clock, wait_clock):
            nc = self.nc
            drain_inst = nc.sync.drain()
            wait_clock.add_sem_waits(
                drain_inst.ins, ScopedClock({None: tick_clock.global_clock})
            )
            nc.all_engine_barrier()
            popped = nc._tile_sem_poison_stack.pop()
            assert popped is self._sem_poison
            sems = list(self.sems.allocated().values())
            sem_nums = [
                s.num if hasattr(s, "num") else int(s) for s in sems
            ]
            ranges = compact_to_ranges(sem_nums)
            for r in ranges:
                assert nc._state.free_isdisjoint(r)
                nc.gpsimd.dma_reset(r)
            # sem_clear lowers to a per-semaphore EVENT_SEMAPHORE train
            # (~115ns per sem, ~6us for the full window); the hardware
            # EVENT_SEMAPHORE_RANGE_CLEAR op zeroes a whole range in one
            # sequencer instruction instead.
            mode = nc.isa.get_enum(
                "NEURON_ISA_TPB_EVENT_SEMAPHORE_CLEAR_MODE"
            ).NEURON_ISA_TPB_EVENT_SEMAPHORE_CLEAR_MODE_SEMAPHORE_ZERO.value
            engines = [nc.gpsimd, nc.sync, nc.scalar, nc.vector, nc.tensor]
            for i, r in enumerate(ranges):
                engines[i % len(engines)].isa(
                    nc.isa.Opcode.NEURON_ISA_TPB_OPCODE_EVENT_SEMAPHORE_RANGE_CLEAR,
                    {
                        "mode": mode,
                        "range_first": r.start,
                        "range_last": r.stop - 1,
                    },
                )
            nc._state.prepend_free_semaphores(sem_nums)
            for poison_set in nc._tile_sem_poison_stack:
                poison_set.update(sem_nums)
            # no second barrier: the engines' own end-of-program epilogues
            # (emitted by the backend) already serialize behind this block
            nc.all_engine_barrier(sem_only=True)

    nc = bacc.Bacc("TRN2", target_bir_lowering=False, debug=False,
                   num_devices=NT * C)

    # spins ship hybrid: slots 0-8 as uint8 (half the bytes; expanded on
    # the vector engine which is otherwise idle early), slots 9-15 as
    # bf16 (no expansion dependency for the late slots)
    su_ext = nc.dram_tensor("su", [P, 9 * BH], u8, kind="ExternalInput").ap()
    sh_ext = nc.dram_tensor("sh", [P, 7 * BH], bf16, kind="ExternalInput").ap()
    jb_ext = nc.dram_tensor("jb", [P, NBLK * P], bf16, kind="ExternalInput").ap()
    # full-partition output (all 128 rows carry the same per-sample
    # partial): tiny single-partition DMAs take a ~10us slow path at the
    # end of a busy kernel, full 128-partition descriptors do not
    OW = 32  # output replication width (full-partition-class DMA, 64KB)
    out_ext = nc.dram_tensor("part", [OW, BH], f32, kind="ExternalOutput").ap()

    with _FastTeardown(nc) as tc:
        with (
            tc.tile_pool(name="persist", bufs=1) as persist,
            tc.tile_pool(name="work", bufs=2) as work,
            tc.tile_pool(name="psum", bufs=1, space="PSUM") as psum_pool,
            tc.tile_pool(name="warmps", bufs=1, space="PSUM") as warm_pool,
        ):
            su_t = persist.tile([P, 9, BH], u8)
            st_t = persist.tile([P, NB, BH], bf16)
            jb_t = persist.tile([P, NBLK, P], bf16)
            ones = persist.tile([P, OW], f32r)
            red_sb = persist.tile([OW, BH], f32)
            warm_src = persist.tile([P, P], bf16)

            nc.vector.memset(warm_src[:], 0.0)
            # f32r memset trips an ISA check; synthesize 1.0s on the DVE
            nc.vector.tensor_scalar(
                ones[:], warm_src[:, :OW], 0.0, 1.0,
                mybir.AluOpType.mult, mybir.AluOpType.add,
            )

            # PE warmup against the HAM clock gate while loads stream in
            warm_ps = warm_pool.tile([P, WARM_N], f32)
            for _ in range(N_WARM):
                nc.tensor.matmul(
                    warm_ps, lhsT=warm_src[:], rhs=warm_src[:, :WARM_N],
                    start=True, stop=True,
                )

            # Input loads on two rings in consumption order (both rings
            # share ~390GB/s once streaming; each dma_start costs ~650ns
            # of descriptor write on its sequencer, so chunks are few with
            # small heads).  The scalar engine does no compute, so its
            # ring has no ACT-table preamble and starts immediately.
            su3 = su_ext.rearrange("p (k b) -> p k b", b=BH)
            sh3 = sh_ext.rearrange("p (k b) -> p k b", b=BH)
            jb3 = jb_ext.rearrange("p (j c) -> p j c", c=P)

            nc.sync.dma_start(out=su_t[:, 0:2], in_=su3[:, 0:2])
            nc.scalar.dma_start(out=jb_t[:, 0:3], in_=jb3[:, 0:3])
            nc.sync.dma_start(out=su_t[:, 2:5], in_=su3[:, 2:5])
            nc.scalar.dma_start(out=jb_t[:, 3:9], in_=jb3[:, 3:9])
            nc.sync.dma_start(out=su_t[:, 5:9], in_=su3[:, 5:9])
            nc.scalar.dma_start(out=jb_t[:, 9:18], in_=jb3[:, 9:18])
            nc.sync.dma_start(out=st_t[:, 9:16], in_=sh3[:, 0:7])
            nc.scalar.dma_start(out=jb_t[:, 18:34], in_=jb3[:, 18:34])

            # expand uint8 {0,1} -> +-1.0 bf16 spins for slots 0-8
            for sl in range(9):
                nc.vector.tensor_scalar(
                    st_t[:, sl], su_t[:, sl], 2.0, -1.0,
                    mybir.AluOpType.mult, mybir.AluOpType.add,
                )

            ps_cols = [
                psum_pool.tile([P, BH], f32, name=f"ps_{k}") for k in range(4)
            ]
            ps_red = psum_pool.tile([OW, BH], f32, name="ps_red")

            # J matmuls with epilogues interleaved.  The column-k reduce
            # matmul is deferred so the PE never stalls on the DVE multiply.
            red_at = {20: 0, 26: 1, 31: 2}
            m_tiles = {}

            def epilogue_mul(k):
                m = work.tile([P, BH], f32r, name="m_col")
                nc.vector.scalar_tensor_tensor(
                    m[:],
                    ps_cols[k][:],
                    1.0,
                    st_t[:, 4 * k],
                    mybir.AluOpType.mult,
                    mybir.AluOpType.mult,
                )
                m_tiles[k] = m

            def red_mm(k):
                nc.tensor.matmul(
                    ps_red,
                    lhsT=ones[:],
                    rhs=m_tiles[k][:],
                    start=(k == 0),
                    stop=(k == 3),
                )

            # Tiny filler matmuls between the early real matmuls keep the
            # PE busy while the first su/jb chunks stream in — gaps there
            # would keep the HAM clock gate at 1.2 GHz for several extra
            # microseconds.
            fillers = {0: 6, 1: 8, 2: 8, 3: 6, 4: 4, 5: 3, 6: 2}

            seen_start = set()
            for j, (k, lam) in enumerate(_BLOCKS):
                nc.tensor.matmul(
                    ps_cols[k],
                    lhsT=jb_t[:, j],
                    rhs=st_t[:, lam],
                    start=(k not in seen_start),
                    stop=(j == _COL_END[k]),
                )
                seen_start.add(k)
                for _ in range(fillers.get(j, 0)):
                    nc.tensor.matmul(
                        warm_ps, lhsT=warm_src[:], rhs=warm_src[:, :WARM_N],
                        start=True, stop=True,
                    )
                if j in _COL_END:
                    epilogue_mul(_COL_END.index(j))
                if j in red_at:
                    red_mm(red_at[j])
            red_mm(3)

            # copy on the vector engine (keeping the scalar engine
            # compute-free avoids its ACT table preamble load entirely);
            # the out DMA issues from the same queue so no cross-engine
            # semaphore hop sits on the critical tail
            nc.vector.tensor_scalar(
                red_sb[:], ps_red[:], 1.0, 0.0,
                mybir.AluOpType.mult, mybir.AluOpType.add,
            )
            nc.sync.dma_start(out=out_ext, in_=red_sb[:])

    nc.compile()
    return nc


def _make_in_maps(state, J):
    bf16 = ml_dtypes.bfloat16
    state = np.asarray(state, dtype=np.float32)
    J = np.asarray(J, dtype=np.float32)

    u_all = state.astype(np.uint8)                   # [B, N] {0,1}
    uT = np.ascontiguousarray(u_all.T).reshape(NB, P, B)
    spins = (state * 2.0 - 1.0).astype(bf16)         # [B, N] exact +-1
    sT = np.ascontiguousarray(spins.T).reshape(NB, P, B)
    A = J + J.T                                      # symmetrized, fp32
    Ab = A.reshape(NB, P, NB, P)

    # J blocks per template c: [34, 128, 128] -> [128, 34*128] bf16
    jb_by_c = []
    for c in range(NT):
        blocks = np.empty((NBLK, P, P), dtype=np.float32)
        for j, (k, lam) in enumerate(_BLOCKS):
            q = (4 * k + c) % NB
            p = (lam + c) % NB
            w = 0.5 if p == q else 1.0
            blocks[j] = Ab[p, :, q, :] * w
        jb_by_c.append(
            np.ascontiguousarray(
                blocks.transpose(1, 0, 2).reshape(P, NBLK * P)
            ).astype(bf16)
        )

    in_maps = []
    placement = []
    for core in range(NT * C):
        c, h = divmod(core, C)
        # local slot lam holds global spin block (lam + c) mod 16;
        # slots 0-8 ship uint8, slots 9-15 ship bf16
        perm = [(lam + c) % NB for lam in range(NB)]
        hs = slice(h * BH, (h + 1) * BH)
        su = np.ascontiguousarray(
            uT[perm[:9]][:, :, hs].transpose(1, 0, 2).reshape(P, 9 * BH)
        )
        sh = np.ascontiguousarray(
            sT[perm[9:]][:, :, hs].transpose(1, 0, 2).reshape(P, 7 * BH)
        )
        in_maps.append({"su": su, "sh": sh, "jb": jb_by_c[c]})
        placement.append((c, h))
    return in_maps, placement


def kernel(state, J):
    from concourse.bass_utils import run_bass_kernel_spmd

    if "nc" not in _cache:
        _cache["nc"] = _build_program()
    nc = _cache["nc"]

    in_maps, placement = _make_in_maps(state, J)
    res = run_bass_kernel_spmd(nc, in_maps, list(range(NT * C)))

    out = np.zeros(B, dtype=np.float32)
    for core, (c, h) in enumerate(placement):
        out[h * BH:(h + 1) * BH] += res.results[core]["part"][0]
    return out
